# revision 1
# baseline (speedup 1.0000x reference)
"""Causal multi-head attention on 8 Trainium2 NeuronCores.

Problem: B=2, S=2048, D=1024, H=16 heads (HD=64), fp32 I/O.
Sharding: batch x head-group. Core c handles batch c//4 and heads
4*(c%4) .. 4*(c%4)+3 (a 256-wide feature slice of Wq/Wk/Wv columns and
Wo rows). Each core writes a partial output projection for its batch;
the host sums the 4 partials per batch and adds the bias.

Device dataflow is fully "feature-major" (transposed) so no transposes
are ever needed on device:
  - host feeds x[b].T as xT [D, S]
  - QT = Wq_g.T @ xT (via matmul(lhsT=Wq chunk, rhs=xT chunk))  [256, S]
  - KT likewise; V in natural token-major layout via lhsT=xT chunks,
    with a ones-column appended per head (V_aug [S, 65]) so the ctx
    matmul's row 64 accumulates the softmax denominator for free
  - scores^T chunks [128 keys, 512 queries] = matmul(lhsT=KT chunk,
    rhs=QT tile) with K=64 contraction; two heads of a pair run as
    row-packed matmuls at base partitions 0/64 (concurrent in the PE)
  - softmax without max-subtraction (inputs are unit-scale gaussians;
    exp cannot overflow): exp on ACT with scale=1/8 fused, causal mask
    applied as a 0/1 multiply only on diagonal-crossing chunks, fully
    masked chunks skipped entirely
  - ctx_aug^T [65, 512] accumulated over key chunks; row 64 = denom
  - normalize: reciprocal of denom row, broadcast across partitions via
    a ones-outer-product matmul, multiply on DVE
  - out^T partial [1024, S] = matmul(lhsT=Wo_g chunk, rhs=ctx^T)
Matmul inputs use dtype float32r (full fp32 storage, ~1.8e-4 matmul
rounding, 4x faster than strict fp32 on the PE).
"""

import numpy as np

B, S, D, H, HD = 2, 2048, 1024, 16, 64
NCORES = 8
GROUPS = 4               # head groups (cores per batch)
HPC = H // GROUPS        # heads per core = 4
DG = HPC * HD            # per-core feature width = 256
P = 128
QT = 512                 # query tile (free dim)
KC = 128                 # key chunk (partition dim)
NQT = S // QT            # 4 query tiles
NKC = S // KC            # 16 key chunks
KCH = D // P             # 8 contraction chunks for projections
MCH = DG // P            # 2 feature chunks per core (= head pairs)
OCH = D // P             # 8 output feature chunks

_compiled = None


def _build(nreps=1):
    import concourse.bass as bass
    import concourse.tile as tile
    from concourse import bacc, mybir

    f32 = mybir.dt.float32
    f32r = mybir.dt.float32r
    EXP = mybir.ActivationFunctionType.Exp

    nc = bacc.Bacc("TRN2", target_bir_lowering=False, debug=False,
                   num_devices=NCORES)

    xT_d = nc.dram_tensor("xT", [D, S], f32r, kind="ExternalInput").ap()
    wq_d = nc.dram_tensor("wq", [D, DG], f32r, kind="ExternalInput").ap()
    wk_d = nc.dram_tensor("wk", [D, DG], f32r, kind="ExternalInput").ap()
    wv_d = nc.dram_tensor("wv", [D, DG], f32r, kind="ExternalInput").ap()
    wo_d = nc.dram_tensor("wo", [DG, D], f32r, kind="ExternalInput").ap()
    g_d = nc.dram_tensor("g", [P, QT + 3 * KC], f32r, kind="ExternalInput").ap()
    ones_d = nc.dram_tensor("ones", [P, HD], f32r, kind="ExternalInput").ap()
    out_d = nc.dram_tensor("outT", [D, S], f32, kind="ExternalOutput").ap()

    with tile.TileContext(nc) as tc:
        with tc.tile_pool(name="const", bufs=1) as const, \
             tc.tile_pool(name="work", bufs=3) as work, \
             tc.tile_pool(name="work2", bufs=2) as work2, \
             tc.tile_pool(name="psA", bufs=2, space="PSUM") as psA, \
             tc.tile_pool(name="psS", bufs=2, space="PSUM") as psS, \
             tc.tile_pool(name="psC", bufs=4, space="PSUM") as psC:

            xT = const.tile([P, KCH, S], f32r, tag="xT")
            wq = const.tile([P, KCH, DG], f32r, tag="wq")
            wk = const.tile([P, KCH, DG], f32r, tag="wk")
            wv = const.tile([P, KCH, DG], f32r, tag="wv")
            wo = const.tile([P, MCH, D], f32r, tag="wo")
            g = const.tile([P, QT + 3 * KC], f32r, tag="g")
            qT = const.tile([P, MCH, S], f32r, tag="qT")
            kT = const.tile([P, MCH, S], f32r, tag="kT")
            v = const.tile([P, NKC, HPC, HD + 1], f32r, tag="v")
            ctx = const.tile([P, MCH, S], f32r, tag="ctx")
            ones = const.tile([P, HD], f32r, tag="ones")

            # ---- input DMAs (weights first so the first projection
            # matmuls can start as soon as xT chunk 0 lands) ----
            for m in range(MCH):
                for w_sb, w_dr in ((wq, wq_d), (wk, wk_d)):
                    nc.sync.dma_start(
                        w_sb[:, :, m * P:(m + 1) * P],
                        w_dr.rearrange("(c p) n -> p c n",
                                       p=P)[:, :, m * P:(m + 1) * P])
            # token-tile-major xT load: tile-0 projections can start after
            # only the first quarter of x has landed; wv/g/ones arrive right
            # after tile 0 so attention(0) isn't starved; wo is only needed
            # by the (deferred) output projections, so it loads last
            def load_xt_tile(t):
                for c in range(KCH):
                    nc.sync.dma_start(
                        xT[:, c, t * QT:(t + 1) * QT],
                        xT_d[c * P:(c + 1) * P, t * QT:(t + 1) * QT])

            load_xt_tile(0)
            nc.sync.dma_start(wv[:], wv_d.rearrange("(c p) n -> p c n", p=P))
            nc.sync.dma_start(g[:], g_d[:])
            nc.sync.dma_start(ones[:], ones_d[:])
            nc.sync.dma_start(
                v[:, :, :, HD:HD + 1],
                ones_d.rearrange("p (a b c) -> p a b c", a=NKC, b=HPC))
            for t in range(1, NQT):
                load_xt_tile(t)
            nc.sync.dma_start(wo[:], wo_d.rearrange("(c p) n -> p c n", p=P))

            from collections import deque

            def gen_proj_qk(w_sb, t_sb, m, t):
                ps = psA.tile([P, QT], f32, tag="mm", name="psq")
                for k in range(KCH):
                    nc.tensor.matmul(
                        ps[:],
                        lhsT=w_sb[:, k, m * P:(m + 1) * P],
                        rhs=xT[:, k, t * QT:(t + 1) * QT],
                        start=(k == 0), stop=(k == KCH - 1))
                    yield
                nc.vector.tensor_scalar_mul(
                    t_sb[:, m, t * QT:(t + 1) * QT], ps[:], 1.0)
                yield

            def gen_proj_v(t):
                ps = psA.tile([P, QT], f32, tag="mm", name="psv")
                for k in range(KCH):
                    nc.tensor.matmul(
                        ps[:, :DG],
                        lhsT=xT[:, k, t * P:(t + 1) * P],
                        rhs=wv[:, k, :],
                        start=(k == 0), stop=(k == KCH - 1))
                    yield
                nc.vector.tensor_scalar_mul(
                    v[:, t, :, 0:HD],
                    ps[:, :DG].rearrange("p (h d) -> p h d", h=HPC), 1.0)
                yield

            def gen_proj_tile(t):
                for m in range(MCH):
                    yield from gen_proj_qk(wq, qT, m, t)
                    yield from gen_proj_qk(wk, kT, m, t)
                for dt_ in range(QT // KC):
                    yield from gen_proj_v(t * (QT // KC) + dt_)

            def gen_outproj(t):
                for m in range(OCH):
                    ps = psA.tile([P, QT], f32, tag="mm", name="pso")
                    for c in range(MCH):
                        nc.tensor.matmul(
                            ps[:],
                            lhsT=wo[:, c, m * P:(m + 1) * P],
                            rhs=ctx[:, c, t * QT:(t + 1) * QT],
                            start=(c == 0), stop=(c == MCH - 1))
                        yield
                    st = work2.tile([P, QT], f32, tag="o", name="st")
                    nc.vector.tensor_copy(st[:], ps[:])
                    nc.sync.dma_start(
                        out_d[m * P:(m + 1) * P, t * QT:(t + 1) * QT], st[:])
                    yield

            def pull(bg, n):
                while n > 0 and bg:
                    try:
                        next(bg[0])
                        n -= 1
                    except StopIteration:
                        bg.popleft()

            def attn_unit(pr, hh, kc, qi, cps):
                nkc = (qi + 1) * (QT // KC)
                off = HD * hh
                diag = kc >= qi * (QT // KC)
                # for a diagonal-crossing chunk, columns below w0 are fully
                # masked: skip them in scores/exp/ctx entirely, and apply the
                # triangular 0/1 mask only to the [P, KC] band at w0
                w0 = KC * (kc - qi * (QT // KC)) if diag else 0
                qlo = qi * QT + w0
                sps = psS.tile([P, QT], f32, tag="s", name="sps")
                nc.tensor.matmul(
                    sps[:, w0:],
                    lhsT=kT[off:off + HD, pr, kc * KC:(kc + 1) * KC],
                    rhs=qT[off:off + HD, pr, qlo:(qi + 1) * QT])
                es = work.tile([P, QT], f32r, tag="e", name="es")
                nc.scalar.activation(es[:, w0:], sps[:, w0:], EXP,
                                     scale=1.0 / np.sqrt(HD))
                if diag:
                    nc.vector.tensor_mul(es[:, w0:w0 + KC],
                                         es[:, w0:w0 + KC],
                                         g[:, QT - KC:QT])
                nc.tensor.matmul(
                    cps[(pr, hh)][:, w0:],
                    lhsT=v[:, kc, 2 * pr + hh, :],
                    rhs=es[:, w0:],
                    start=(kc == 0), stop=(kc == nkc - 1))

            def attn_norm(pr, hh, qi, cps, bg):
                qs = slice(qi * QT, (qi + 1) * QT)
                cp = cps[(pr, hh)]
                # evict the denominator row, broadcast it across partitions
                # via a ones-outer-product matmul, then one fast reciprocal
                # of the broadcast followed by the normalize multiply
                rt = work2.tile([HD + 1, QT], f32r, tag="r", name="rt")
                nc.vector.tensor_scalar_mul(rt[HD:HD + 1, :],
                                            cp[HD:HD + 1, :], 1.0)
                rbp = psS.tile([P, QT], f32, tag="s", name="rbp")
                nc.tensor.matmul(rbp[:HD, :],
                                 lhsT=ones[HD:HD + 1, :],
                                 rhs=rt[HD:HD + 1, :])
                rbs = work2.tile([HD, QT], f32, tag="rb", name="rbs")
                nc.vector.reciprocal_approx_fast(rbs[:], rbp[0:HD, :])
                off = HD * hh
                nc.vector.tensor_mul(
                    ctx[off:off + HD, pr, qs], cp[0:HD, :], rbs[:])

            N_PROJ_Q = MCH * 2 * (KCH + 1) + (QT // KC) * (KCH + 1)
            N_OUT_Q = OCH * (MCH + 1)

            def phases():
                # Software-pipelined emission. Emission order IS program
                # order for Tile, so a tile's projections must be fully
                # emitted before any attention unit that reads them; we
                # spread proj(t+1) + outproj(t-1) quanta evenly across
                # attention(t)'s pull points and force-drain at the tile
                # boundary.
                bgP = deque()   # proj work: must drain by tile boundary
                bgO = deque()   # outproj work: no boundary deadline
                remaining = [0]

                def pull_n(n):
                    n = min(n, remaining[0])
                    remaining[0] -= n
                    pull(bgP, n)

                for _ in gen_proj_tile(0):
                    pass
                for t in range(NQT):
                    if t + 1 < NQT:
                        bgP.append(gen_proj_tile(t + 1))
                        remaining[0] += N_PROJ_Q
                    last = t == NQT - 1
                    nkc = (t + 1) * (QT // KC)
                    points = MCH * (nkc + 2)
                    for pr in range(MCH):
                        cps = {(pr, hh): psC.tile([HD + 1, QT], f32,
                                                  tag="ctx",
                                                  name=f"ctx_{t}_{pr}_{hh}")
                               for hh in range(2)}
                        for kc in range(nkc):
                            for hh in range(2):
                                attn_unit(pr, hh, kc, t, cps)
                            pull_n(-(-remaining[0] // max(points - 2, 1)))
                            if last:
                                pull(bgO, 4)
                            points -= 1
                        for hh in range(2):
                            attn_norm(pr, hh, t, cps, bgP)
                            pull_n(-(-remaining[0] // max(points - 2, 1)))
                            if last:
                                pull(bgO, 4)
                            points -= 1
                    pull(bgP, 10 ** 9)  # safety drain: emission-order deps
                    remaining[0] = 0
                    bgO.append(gen_outproj(t))
                pull(bgO, 10 ** 9)

            for _ in range(nreps):
                phases()

    nc.compile()
    return nc


def _mask():
    # G[k, j] = 1.0 iff k <= j - (QT - KC); slice [*, goff:goff+QT] gives
    # the 0/1 causal mask for a key chunk at relative offset crel within
    # a query tile: keep iff k + KC*crel <= q.
    j = np.arange(QT + 3 * KC)[None, :]
    k = np.arange(P)[:, None]
    return (k <= j - (QT - KC)).astype(np.float32)


def _in_maps(x, Wq, Wk, Wv, Wo):
    G = _mask()
    maps = []
    for c in range(NCORES):
        b, gidx = divmod(c, GROUPS)
        sl = slice(gidx * DG, (gidx + 1) * DG)
        maps.append({
            "xT": np.ascontiguousarray(x[b].T),
            "wq": np.ascontiguousarray(Wq[:, sl]),
            "wk": np.ascontiguousarray(Wk[:, sl]),
            "wv": np.ascontiguousarray(Wv[:, sl]),
            "wo": np.ascontiguousarray(Wo[sl, :]),
            "g": G,
            "ones": np.ones((P, HD), dtype=np.float32),
        })
    return maps


def kernel(x, Wq, Wk, Wv, Wo, bo):
    global _compiled
    from concourse.bass_utils import run_bass_kernel_spmd

    x = np.asarray(x, dtype=np.float32)
    Wq = np.asarray(Wq, dtype=np.float32)
    Wk = np.asarray(Wk, dtype=np.float32)
    Wv = np.asarray(Wv, dtype=np.float32)
    Wo = np.asarray(Wo, dtype=np.float32)
    bo = np.asarray(bo, dtype=np.float32)

    if _compiled is None:
        _compiled = _build()
    nc = _compiled

    res = run_bass_kernel_spmd(nc, _in_maps(x, Wq, Wk, Wv, Wo),
                               list(range(NCORES)))
    out = np.zeros((B, S, D), dtype=np.float32)
    for c in range(NCORES):
        out[c // GROUPS] += res.results[c]["outT"].T
    out += bo
    return out



# revision 68
# speedup vs baseline: 1.3783x; 1.3783x over previous
"""Causal multi-head attention on 8 Trainium2 NeuronCores.

Problem: B=2, S=2048, D=1024, H=16 heads (HD=64), fp32 I/O.
Sharding: batch x head-group. Core c handles batch c//4 and heads
4*(c%4) .. 4*(c%4)+3 (a 256-wide feature slice of Wq/Wk/Wv columns and
Wo rows). Each core writes a partial output projection for its batch;
the host sums the 4 partials per batch and adds the bias.

v2 changes vs the f32r baseline (cost-model 182us -> 132us):
  - All device dataflow is bf16 (PSUM accumulation stays fp32): halves
    DMA traffic, fixes the <256-wide fp32r matmul penalty, and enables
    2x DVE modes. Host converts inputs/outputs.
  - ctx accumulation is token-major: for each 128-query subchunk,
    matmul(lhsT=es[:, qsub], rhs=v_chunk[128k, 65]) accumulates
    ctx[128q, 65] (65 PE rows/chunk instead of 512-wide ctx rows,
    ~2x less PE time for the attention*V stage; column 64 is the
    softmax denominator via the ones-column of v). Only the first
    matmul into a psC bank carries start=True: start_tensor_calc
    zeroes the whole 2KB PSUM zero-region, so the other qsub groups
    initialize through the lazy pending-zero bytes.
  - normalization via per-partition TensorScalar (denominator is a
    column now), killing the ones-broadcast matmuls; normalized
    token-major ctx returns to feature-major via the DMA transpose
    XBAR (no engine time) for the output projection.
  - scores for two adjacent key chunks land in one [128, 2, 512] PSUM
    pool slot so a single exp covers both, halving ACT instruction
    overheads. Pool-managed slots keep the write-after-read deps
    per-slot (a single rotating tensor serializes on the coalesced
    interval tracking).
  - per-tile qT/kT/ctx and per-chunk v tiles keep the dependency
    tracking precise; fully masked (chunk, qsub) pairs are skipped.
  - software pipelining: the ctx matmuls of a unit are emitted two
    units later (their es is complete, so they never clog the PE wait
    queue), the normalize/transpose posts one unit after that, both
    carried across pr-group and tile boundaries; causal masks are
    emitted at flush time just before their ctx matmuls.
  - Q/K/V projections of the next tile and deferred output
    projections drain through pacing queues to fill the PE during the
    activation-bound attention stretches; output projections are held
    back until late tiles (and their evictions split DVE/ACT on the
    last tile) to cover the tail.
"""

import numpy as np

B, S, D, H, HD = 2, 2048, 1024, 16, 64
NCORES = 8
GROUPS = 4               # head groups (cores per batch)
HPC = H // GROUPS        # heads per core = 4
DG = HPC * HD            # per-core feature width = 256
P = 128
QT = 512                 # query tile (free dim)
KC = 128                 # key chunk (partition dim)
NQT = S // QT            # 4 query tiles
NKC = S // KC            # 16 key chunks
KCH = D // P             # 8 contraction chunks for projections
MCH = DG // P            # 2 feature chunks per core (= head pairs)
OCH = D // P             # 8 output feature chunks
QS = QT // P             # 4 query subchunks per tile
NB = 4                   # scores PSUM banks (manual rotation)

DRAM_SPECS = [
    ("xT", [D, S], "bfloat16", "in"),
    ("wq", [D, DG], "bfloat16", "in"),
    ("wk", [D, DG], "bfloat16", "in"),
    ("wv", [D, DG], "bfloat16", "in"),
    ("wo", [DG, D], "bfloat16", "in"),
    ("g", [P, KC], "bfloat16", "in"),
    ("outT", [D, S], "bfloat16", "out"),
]

_compiled = None


def _build(nreps=1):
    import concourse.bass as bass
    import concourse.tile as tile
    from concourse import bacc, mybir

    f32 = mybir.dt.float32
    bf16 = mybir.dt.bfloat16
    EXP = mybir.ActivationFunctionType.Exp

    nc = bacc.Bacc("TRN2", target_bir_lowering=False, debug=False,
                   num_devices=NCORES)

    xT_d = nc.dram_tensor("xT", [D, S], bf16, kind="ExternalInput").ap()
    wq_d = nc.dram_tensor("wq", [D, DG], bf16, kind="ExternalInput").ap()
    wk_d = nc.dram_tensor("wk", [D, DG], bf16, kind="ExternalInput").ap()
    wv_d = nc.dram_tensor("wv", [D, DG], bf16, kind="ExternalInput").ap()
    wo_d = nc.dram_tensor("wo", [DG, D], bf16, kind="ExternalInput").ap()
    g_d = nc.dram_tensor("g", [P, KC], bf16, kind="ExternalInput").ap()
    out_d = nc.dram_tensor("outT", [D, S], bf16, kind="ExternalOutput").ap()

    with tile.TileContext(nc) as tc:
        with tc.tile_pool(name="const", bufs=1) as const, \
             tc.tile_pool(name="work", bufs=8) as work, \
             tc.tile_pool(name="work2", bufs=2) as work2, \
             tc.tile_pool(name="psSp", bufs=2, space="PSUM") as psSp, \
             tc.tile_pool(name="psA", bufs=2, space="PSUM") as psA, \
             tc.tile_pool(name="psC", bufs=2, space="PSUM") as psC:

            xT = [const.tile([P, KCH, QT], bf16, tag=f"xT{t}",
                             name=f"xT{t}") for t in range(NQT)]
            wq = const.tile([P, KCH, DG], bf16, tag="wq")
            wk = const.tile([P, KCH, DG], bf16, tag="wk")
            wv = const.tile([P, KCH, DG], bf16, tag="wv")
            wo = const.tile([P, MCH, D], bf16, tag="wo")
            g = const.tile([P, KC], bf16, tag="g")
            qT = [const.tile([P, MCH, QT], bf16, tag=f"qT{t}",
                             name=f"qT{t}") for t in range(NQT)]
            kT = [const.tile([P, MCH, QT], bf16, tag=f"kT{t}",
                             name=f"kT{t}") for t in range(NQT)]
            v = [const.tile([P, HPC, HD + 1], bf16, tag=f"v{c}",
                            name=f"v{c}") for c in range(NKC)]
            ctx = [const.tile([P, MCH, QT], bf16, tag=f"ctx{t}",
                              name=f"ctx{t}") for t in range(NQT)]


            # denominator ones-column of v: no DMA needed
            for c in range(NKC):
                nc.gpsimd.memset(v[c][:, :, HD:HD + 1], 1.0)

            # ---- input DMAs: weights for QK first, then xT tile 0 so
            # the first projection matmuls start ASAP; wv/g next so
            # attention(0) isn't starved; remaining xT; wo last (only
            # needed by the deferred output projections).
            nc.sync.dma_start(wq[:], wq_d.rearrange("(c p) n -> p c n", p=P))
            nc.sync.dma_start(wk[:], wk_d.rearrange("(c p) n -> p c n", p=P))
            for c in range(KCH):
                nc.sync.dma_start(xT[0][:, c, :], xT_d[c * P:(c + 1) * P, 0:QT])
            nc.sync.dma_start(wv[:], wv_d.rearrange("(c p) n -> p c n", p=P))
            nc.sync.dma_start(g[:], g_d[:])
            for t in range(1, NQT):
                nc.sync.dma_start(
                    xT[t][:],
                    xT_d.rearrange("(c p) n -> p c n",
                                   p=P)[:, :, t * QT:(t + 1) * QT])
            nc.sync.dma_start(wo[:], wo_d.rearrange("(c p) n -> p c n", p=P))

            from collections import deque

            def gen_proj_qk(w_sb, t_sb, m, t):
                ps = psA.tile([P, QT], f32, tag="mm", name="psq")
                for k in range(KCH):
                    nc.tensor.matmul(
                        ps[:],
                        lhsT=w_sb[:, k, m * P:(m + 1) * P],
                        rhs=xT[t][:, k, :],
                        start=(k == 0), stop=(k == KCH - 1))
                    yield
                nc.vector.tensor_scalar_mul(
                    t_sb[t][:, m, :], ps[:], 1.0)
                yield

            def gen_proj_v(c):
                ps = psA.tile([P, QT], f32, tag="mm", name="psv")
                tv, dt_ = divmod(c, QS)
                for k in range(KCH):
                    nc.tensor.matmul(
                        ps[:, :DG],
                        lhsT=xT[tv][:, k, dt_ * P:(dt_ + 1) * P],
                        rhs=wv[:, k, :],
                        start=(k == 0), stop=(k == KCH - 1))
                    yield
                nc.vector.tensor_scalar_mul(
                    v[c][:, :, 0:HD],
                    ps[:, :DG].rearrange("p (h d) -> p h d", h=HPC), 1.0)
                yield

            def gen_proj_qk_tile(t):
                for m in range(MCH):
                    yield from gen_proj_qk(wq, qT, m, t)
                    yield from gen_proj_qk(wk, kT, m, t)

            def gen_proj_v4(t):
                for dt_ in range(QS):
                    yield from gen_proj_v(t * QS + dt_)

            def gen_outproj(t):
                for m in range(OCH):
                    ps = psA.tile([P, QT], f32, tag="mm", name="pso")
                    for c in range(MCH):
                        nc.tensor.matmul(
                            ps[:],
                            lhsT=wo[:, c, m * P:(m + 1) * P],
                            rhs=ctx[t][:, c, :],
                            start=(c == 0), stop=(c == MCH - 1))
                        yield
                    st = work2.tile([P, QT], bf16, tag="o", name="st",
                                    bufs=6)
                    if t == NQT - 1 and m % 2 == 1:
                        # the activation engine is idle during the final
                        # output projection: split the evictions
                        nc.scalar.copy(st[:], ps[:])
                    else:
                        nc.vector.tensor_scalar_mul(st[:], ps[:], 1.0)
                    nc.sync.dma_start(
                        out_d[m * P:(m + 1) * P, t * QT:(t + 1) * QT], st[:])
                    yield

            SC = 1.0 / np.sqrt(HD)

            pend = deque()    # ctx matmuls, lag 3 units (carried
            pend2 = deque()   # across pr/tile boundaries so forced
                              # flushes never pile up unready waiters)

            def flush_one():
                if pend2:
                    pend2.popleft()()
                if pend:
                    ctx_c, post_c = pend.popleft()
                    ctx_c()
                    if post_c is not None:
                        pend2.append(post_c)

            def flush_all():
                while pend or pend2:
                    flush_one()

            def attn_tile(t, pulls):
                # One unit per (key chunk, head). Scores take one PSUM
                # bank each from a 4-slot pool, so a unit's scores
                # conflict only with the exp four units back. ctx
                # matmuls lag three units so their es input is complete
                # before they reach the PE queue; normalize posts lag
                # one further unit.
                nkc = (t + 1) * QS
                for pr in range(MCH):
                    cps = {hh: psC.tile([P, QS, HD + 1], f32, tag="ctx",
                                        name=f"c{t}_{pr}_{hh}")
                           for hh in range(2)}
                    rec = {hh: work2.tile([P, QS], f32, tag="rc",
                                          name=f"rc{hh}", bufs=4)
                           for hh in range(2)}
                    tms = {}

                    def make_ctx(es, kc, hh, r, cps_=None):
                        cps_ = cps
                        head = 2 * pr + hh
                        jlo = 0 if r is None else r

                        def emit():
                            if r is not None:
                                nc.vector.tensor_mul(
                                    es[:, 0, KC * r:KC * r + KC],
                                    es[:, 0, KC * r:KC * r + KC], g[:])
                            for j in range(jlo, QS):
                                nc.tensor.matmul(
                                    cps_[hh][:, j, :],
                                    lhsT=es[:, 0, j * P:(j + 1) * P],
                                    rhs=v[kc][:, head, :],
                                    start=(kc == 0 and j == 0),
                                    stop=(r is not None and j == r),
                                    skip_group_check=True)
                        return emit

                    def make_ctx_pair(es, i0, hh, cps_=None):
                        cps_ = cps
                        head = 2 * pr + hh

                        def emit():
                            for h2 in range(2):
                                kc = i0 + h2
                                for j in range(QS):
                                    nc.tensor.matmul(
                                        cps_[hh][:, j, :],
                                        lhsT=es[:, h2,
                                                j * P:(j + 1) * P],
                                        rhs=v[kc][:, head, :],
                                        start=(kc == 0 and j == 0),
                                        stop=False,
                                        skip_group_check=True)
                        return emit

                    def make_post(hh, r, cps_=None, rec_=None, tms_=None,
                                  pr_=None):
                        cps_, rec_, tms_, pr_ = cps, rec, tms, pr

                        def post():
                            # qsub r of head (pr, hh) is complete:
                            # normalize (denominator at column HD); when
                            # both heads are done, ship the block back
                            # feature-major via the DMA transpose XBAR
                            if hh == 0:
                                tms_[r] = work2.tile([P, 2, HD], bf16,
                                                     tag="tm", name="tm",
                                                     bufs=6)
                            nc.vector.reciprocal_approx_fast(
                                rec_[hh][:, r:r + 1],
                                cps_[hh][:, r, HD:HD + 1])
                            nc.vector.tensor_scalar_mul(
                                tms_[r][:, hh, :], cps_[hh][:, r, 0:HD],
                                rec_[hh][:, r:r + 1])
                            if hh == 1:
                                nc.sync.dma_start_transpose(
                                    ctx[t][:, pr_, r * P:(r + 1) * P],
                                    tms_[r].rearrange("p a b -> p (a b)"))
                        return post

                    # non-diagonal chunks in pairs: two scores
                    # matmuls into one 2-bank slot, a single exp over
                    # both halves
                    for i0 in range(0, 4 * t, 2):
                        for hh in range(2):
                            off = HD * hh
                            sps = psSp.tile([P, 2, QT], f32, tag="sc",
                                            name="sps")
                            for h2 in range(2):
                                kc = i0 + h2
                                kcl, kco = divmod(kc * KC, QT)
                                nc.tensor.matmul(
                                    sps[:, h2, :],
                                    lhsT=kT[kcl][off:off + HD, pr,
                                                 kco:kco + KC],
                                    rhs=qT[t][off:off + HD, pr, :])
                            pulls()
                            es = work.tile([P, 2, QT], bf16, tag="e",
                                           name="es")
                            nc.scalar.activation(
                                es.rearrange("p a b -> p (a b)"),
                                sps.rearrange("p a b -> p (a b)"),
                                EXP, scale=SC)
                            pend.append((make_ctx_pair(es, i0, hh),
                                         None))
                            if len(pend) > 2:
                                flush_one()
                    # diagonal chunks: single, query-trimmed, masked
                    for r in range(QS):
                        kc = 4 * t + r
                        w0 = KC * r
                        kcl, kco = divmod(kc * KC, QT)
                        for hh in range(2):
                            off = HD * hh
                            sps = psSp.tile([P, 2, QT], f32, tag="sc",
                                            name="sps")
                            nc.tensor.matmul(
                                sps[:, 0, w0:],
                                lhsT=kT[kcl][off:off + HD, pr,
                                             kco:kco + KC],
                                rhs=qT[t][off:off + HD, pr, w0:])
                            pulls()
                            es = work.tile([P, 2, QT], bf16, tag="e",
                                           name="es")
                            nc.scalar.activation(es[:, 0, w0:],
                                                 sps[:, 0, w0:], EXP,
                                                 scale=SC)
                            pend.append((make_ctx(es, kc, hh, r),
                                         make_post(hh, r)))
                            if len(pend) > 2:
                                flush_one()

            N_QK_Q = MCH * 2 * (KCH + 1)
            N_V_Q = QS * (KCH + 1)

            def pull(bg, n):
                pulled = 0
                while n > 0 and bg:
                    try:
                        next(bg[0])
                        n -= 1
                        pulled += 1
                    except StopIteration:
                        bg.popleft()
                return pulled

            def phases():
                # Three filler queues keep the PE fed during the
                # latency-bound attention stretches:
                #   bgV  - V projection of the CURRENT tile's key chunks;
                #          hard deadline: before the tile's diagonal
                #          (singles) phase reads them
                #   bgQK - Q/K projection of the NEXT tile; deadline:
                #          end of the current tile
                #   bgO  - output projections; no deadline, so they are
                #          deferred to the last (most starved) tile
                bgV = deque()
                bgQK = deque()
                bgO = deque()
                remV = [0]
                remQK = [0]

                # prologue: just enough of proj(0) for att(0) pr=0 (the
                # m=0 half of Q/K plus v chunks 0-3); the m=1 half drains
                # via bgQK during att(0) well before pr=1 needs it
                for _ in gen_proj_qk(wq, qT, 0, 0):
                    pass
                for _ in gen_proj_qk(wk, kT, 0, 0):
                    pass
                for _ in gen_proj_v4(0):
                    pass

                def gen_proj0_m1():
                    yield from gen_proj_qk(wq, qT, 1, 0)
                    yield from gen_proj_qk(wk, kT, 1, 0)

                bgQK.append(gen_proj0_m1())
                remQK[0] += 2 * (KCH + 1)

                RATE_O = {0: 0, 1: 0, 2: 1, 3: 1}
                for t in range(NQT):
                    if t + 1 < NQT:
                        bgQK.append(gen_proj_qk_tile(t + 1))
                        remQK[0] += N_QK_Q
                    if t >= 1:
                        # V projection of THIS tile's diagonal key
                        # chunks: drains over the pr=0 pairs phase,
                        # finishing before the singles read it
                        bgV.append(gen_proj_v4(t))
                        remV[0] += N_V_Q
                    points = [MCH * 2 * (2 * t + QS)]
                    total = points[0]
                    vpts = [4 * t if t >= 1 else 2]

                    def pulls():
                        if vpts[0] > 0:
                            n = -(-remV[0] // vpts[0])
                        else:
                            n = remV[0]
                        n = min(n, remV[0])
                        remV[0] -= n
                        pull(bgV, n)
                        n = -(-remQK[0] // max(points[0], 1))
                        n = min(n, remQK[0])
                        remQK[0] -= n
                        pull(bgQK, n)
                        # outproj(t-1) reads ctx written by transposes
                        # that may still sit in the carried pipeline for
                        # the first few units of this tile; pulling bgO
                        # before they are emitted would hide the RAW dep
                        if total - points[0] > 6:
                            extra = 2 if (not bgV and not bgQK) else 0
                            pull(bgO, RATE_O[t] + extra)
                        points[0] -= 1
                        vpts[0] -= 1

                    attn_tile(t, pulls)
                    pull(bgV, 10 ** 9)   # drain: emission-order deps
                    pull(bgQK, 10 ** 9)
                    remV[0] = 0
                    remQK[0] = 0
                    bgO.append(gen_outproj(t))
                flush_all()
                pull(bgO, 10 ** 9)

            for _ in range(nreps):
                phases()

    nc.compile()
    return nc


def _mask():
    # mask[k, c] = 1.0 iff key row k <= query col c (within the
    # diagonal 128x128 band of a diagonal-crossing chunk)
    k = np.arange(P)[:, None]
    c = np.arange(KC)[None, :]
    return (k <= c).astype(np.float32)


def _in_maps(x, Wq, Wk, Wv, Wo):
    import ml_dtypes
    bf = ml_dtypes.bfloat16
    G = _mask().astype(bf)
    maps = []
    for c in range(NCORES):
        b, gidx = divmod(c, GROUPS)
        sl = slice(gidx * DG, (gidx + 1) * DG)
        maps.append({
            "xT": np.ascontiguousarray(x[b].T).astype(bf),
            "wq": np.ascontiguousarray(Wq[:, sl]).astype(bf),
            "wk": np.ascontiguousarray(Wk[:, sl]).astype(bf),
            "wv": np.ascontiguousarray(Wv[:, sl]).astype(bf),
            "wo": np.ascontiguousarray(Wo[sl, :]).astype(bf),
            "g": G,
        })
    return maps


def kernel(x, Wq, Wk, Wv, Wo, bo):
    global _compiled
    from concourse.bass_utils import run_bass_kernel_spmd

    x = np.asarray(x, dtype=np.float32)
    Wq = np.asarray(Wq, dtype=np.float32)
    Wk = np.asarray(Wk, dtype=np.float32)
    Wv = np.asarray(Wv, dtype=np.float32)
    Wo = np.asarray(Wo, dtype=np.float32)
    bo = np.asarray(bo, dtype=np.float32)

    if _compiled is None:
        _compiled = _build()
    nc = _compiled

    res = run_bass_kernel_spmd(nc, _in_maps(x, Wq, Wk, Wv, Wo),
                               list(range(NCORES)))
    out = np.zeros((B, S, D), dtype=np.float32)
    for c in range(NCORES):
        out[c // GROUPS] += res.results[c]["outT"].T.astype(np.float32)
    out += bo
    return out


# revision 78
# speedup vs baseline: 1.3847x; 1.0046x over previous
"""Causal multi-head attention on 8 Trainium2 NeuronCores.

Problem: B=2, S=2048, D=1024, H=16 heads (HD=64), fp32 I/O.
Sharding: batch x head-group. Core c handles batch c//4 and heads
4*(c%4) .. 4*(c%4)+3 (a 256-wide feature slice of Wq/Wk/Wv columns and
Wo rows). Each core writes a partial output projection for its batch;
the host sums the 4 partials per batch and adds the bias.

v2 changes vs the f32r baseline (cost-model 182us -> 132us):
  - All device dataflow is bf16 (PSUM accumulation stays fp32): halves
    DMA traffic, fixes the <256-wide fp32r matmul penalty, and enables
    2x DVE modes. Host converts inputs/outputs.
  - ctx accumulation is token-major: for each 128-query subchunk,
    matmul(lhsT=es[:, qsub], rhs=v_chunk[128k, 65]) accumulates
    ctx[128q, 65] (65 PE rows/chunk instead of 512-wide ctx rows,
    ~2x less PE time for the attention*V stage; column 64 is the
    softmax denominator via the ones-column of v). Only the first
    matmul into a psC bank carries start=True: start_tensor_calc
    zeroes the whole 2KB PSUM zero-region, so the other qsub groups
    initialize through the lazy pending-zero bytes.
  - normalization via per-partition TensorScalar (denominator is a
    column now), killing the ones-broadcast matmuls; normalized
    token-major ctx returns to feature-major via the DMA transpose
    XBAR (no engine time) for the output projection.
  - scores for two adjacent key chunks land in one [128, 2, 512] PSUM
    pool slot so a single exp covers both, halving ACT instruction
    overheads. Pool-managed slots keep the write-after-read deps
    per-slot (a single rotating tensor serializes on the coalesced
    interval tracking).
  - per-tile qT/kT/ctx and per-chunk v tiles keep the dependency
    tracking precise; fully masked (chunk, qsub) pairs are skipped.
  - software pipelining: the ctx matmuls of a unit are emitted two
    units later (their es is complete, so they never clog the PE wait
    queue), the normalize/transpose posts one unit after that, both
    carried across pr-group and tile boundaries; causal masks are
    emitted at flush time just before their ctx matmuls.
  - Q/K/V projections of the next tile and deferred output
    projections drain through pacing queues to fill the PE during the
    activation-bound attention stretches; output projections are held
    back until late tiles (and their evictions split DVE/ACT on the
    last tile) to cover the tail.
"""

import numpy as np

B, S, D, H, HD = 2, 2048, 1024, 16, 64
NCORES = 8
GROUPS = 4               # head groups (cores per batch)
HPC = H // GROUPS        # heads per core = 4
DG = HPC * HD            # per-core feature width = 256
P = 128
QT = 512                 # query tile (free dim)
KC = 128                 # key chunk (partition dim)
NQT = S // QT            # 4 query tiles
NKC = S // KC            # 16 key chunks
KCH = D // P             # 8 contraction chunks for projections
MCH = DG // P            # 2 feature chunks per core (= head pairs)
OCH = D // P             # 8 output feature chunks
QS = QT // P             # 4 query subchunks per tile
NB = 4                   # scores PSUM banks (manual rotation)

DRAM_SPECS = [
    ("xT", [D, S], "bfloat16", "in"),
    ("wq", [D, DG], "bfloat16", "in"),
    ("wk", [D, DG], "bfloat16", "in"),
    ("wv", [D, DG], "bfloat16", "in"),
    ("wo", [DG, D], "bfloat16", "in"),
    ("g", [P, KC], "bfloat16", "in"),
    ("outT", [D, S], "bfloat16", "out"),
]

_compiled = None


def _build(nreps=1):
    import concourse.bass as bass
    import concourse.tile as tile
    from concourse import bacc, mybir

    f32 = mybir.dt.float32
    bf16 = mybir.dt.bfloat16
    EXP = mybir.ActivationFunctionType.Exp

    nc = bacc.Bacc("TRN2", target_bir_lowering=False, debug=False,
                   num_devices=NCORES)

    xT_d = nc.dram_tensor("xT", [D, S], bf16, kind="ExternalInput").ap()
    wq_d = nc.dram_tensor("wq", [D, DG], bf16, kind="ExternalInput").ap()
    wk_d = nc.dram_tensor("wk", [D, DG], bf16, kind="ExternalInput").ap()
    wv_d = nc.dram_tensor("wv", [D, DG], bf16, kind="ExternalInput").ap()
    wo_d = nc.dram_tensor("wo", [DG, D], bf16, kind="ExternalInput").ap()
    g_d = nc.dram_tensor("g", [P, KC], bf16, kind="ExternalInput").ap()
    out_d = nc.dram_tensor("outT", [D, S], bf16, kind="ExternalOutput").ap()

    with tile.TileContext(nc) as tc:
        with tc.tile_pool(name="const", bufs=1) as const, \
             tc.tile_pool(name="work", bufs=8) as work, \
             tc.tile_pool(name="work2", bufs=2) as work2, \
             tc.tile_pool(name="psSp", bufs=2, space="PSUM") as psSp, \
             tc.tile_pool(name="psA", bufs=2, space="PSUM") as psA, \
             tc.tile_pool(name="psC", bufs=2, space="PSUM") as psC:

            xT = [const.tile([P, KCH, QT], bf16, tag=f"xT{t}",
                             name=f"xT{t}") for t in range(NQT)]
            wq = const.tile([P, KCH, DG], bf16, tag="wq")
            wk = const.tile([P, KCH, DG], bf16, tag="wk")
            wv = const.tile([P, KCH, DG], bf16, tag="wv")
            wo = const.tile([P, MCH, D], bf16, tag="wo")
            g = const.tile([P, KC], bf16, tag="g")
            qT = [const.tile([P, MCH, QT], bf16, tag=f"qT{t}",
                             name=f"qT{t}") for t in range(NQT)]
            kT = [const.tile([P, MCH, QT], bf16, tag=f"kT{t}",
                             name=f"kT{t}") for t in range(NQT)]
            v = [const.tile([P, HPC, HD + 1], bf16, tag=f"v{c}",
                            name=f"v{c}") for c in range(NKC)]
            ctx = [const.tile([P, MCH, QT], bf16, tag=f"ctx{t}",
                              name=f"ctx{t}") for t in range(NQT)]


            # warmup while the input DMAs are in flight: back-to-back
            # dummy matmuls ramp the PE p-state to full clock (3us of
            # continuous execution) and a dummy exp pulls the activation
            # table load off the critical path
            wtile = const.tile([P, QT], bf16, tag="wu")
            nc.gpsimd.memset(wtile[:], 0.0)

            # denominator ones-column of v: no DMA needed
            for c in range(NKC):
                nc.gpsimd.memset(v[c][:, :, HD:HD + 1], 1.0)

            # ---- input DMAs: weights for QK first, then xT tile 0 so
            # the first projection matmuls start ASAP; wv/g next so
            # attention(0) isn't starved; remaining xT; wo last (only
            # needed by the deferred output projections).
            nc.sync.dma_start(wq[:], wq_d.rearrange("(c p) n -> p c n", p=P))
            nc.sync.dma_start(wk[:], wk_d.rearrange("(c p) n -> p c n", p=P))
            for c in range(KCH):
                nc.sync.dma_start(xT[0][:, c, :], xT_d[c * P:(c + 1) * P, 0:QT])
            nc.sync.dma_start(wv[:], wv_d.rearrange("(c p) n -> p c n", p=P))
            nc.sync.dma_start(g[:], g_d[:])
            for t in range(1, NQT):
                nc.sync.dma_start(
                    xT[t][:],
                    xT_d.rearrange("(c p) n -> p c n",
                                   p=P)[:, :, t * QT:(t + 1) * QT])
            nc.sync.dma_start(wo[:], wo_d.rearrange("(c p) n -> p c n", p=P))

            wps = psA.tile([P, QT], f32, tag="mm", name="wps")
            for _ in range(8):
                nc.tensor.matmul(wps[:], lhsT=wtile[:, 0:P], rhs=wtile[:],
                                 start=True, stop=True)
            wes = work.tile([P, QT], bf16, tag="e", name="wes")
            nc.scalar.activation(wes[:], wps[:],
                                 mybir.ActivationFunctionType.Exp,
                                 scale=1.0)

            from collections import deque

            def gen_proj_qk(w_sb, t_sb, m, t):
                ps = psA.tile([P, QT], f32, tag="mm", name="psq")
                for k in range(KCH):
                    nc.tensor.matmul(
                        ps[:],
                        lhsT=w_sb[:, k, m * P:(m + 1) * P],
                        rhs=xT[t][:, k, :],
                        start=(k == 0), stop=(k == KCH - 1))
                    yield
                nc.vector.tensor_scalar_mul(
                    t_sb[t][:, m, :], ps[:], 1.0)
                yield

            def gen_proj_v(c):
                ps = psA.tile([P, QT], f32, tag="mm", name="psv")
                tv, dt_ = divmod(c, QS)
                for k in range(KCH):
                    nc.tensor.matmul(
                        ps[:, :DG],
                        lhsT=xT[tv][:, k, dt_ * P:(dt_ + 1) * P],
                        rhs=wv[:, k, :],
                        start=(k == 0), stop=(k == KCH - 1))
                    yield
                nc.vector.tensor_scalar_mul(
                    v[c][:, :, 0:HD],
                    ps[:, :DG].rearrange("p (h d) -> p h d", h=HPC), 1.0)
                yield

            def gen_proj_qk_tile(t):
                for m in range(MCH):
                    yield from gen_proj_qk(wq, qT, m, t)
                    yield from gen_proj_qk(wk, kT, m, t)

            def gen_proj_v4(t):
                for dt_ in range(QS):
                    yield from gen_proj_v(t * QS + dt_)

            def gen_outproj(t):
                for m in range(OCH):
                    ps = psA.tile([P, QT], f32, tag="mm", name="pso")
                    for c in range(MCH):
                        nc.tensor.matmul(
                            ps[:],
                            lhsT=wo[:, c, m * P:(m + 1) * P],
                            rhs=ctx[t][:, c, :],
                            start=(c == 0), stop=(c == MCH - 1))
                        yield
                    st = work2.tile([P, QT], bf16, tag="o", name="st",
                                    bufs=6)
                    if t == NQT - 1 and m % 2 == 1:
                        # the activation engine is idle during the final
                        # output projection: split the evictions
                        nc.scalar.copy(st[:], ps[:])
                    else:
                        nc.vector.tensor_scalar_mul(st[:], ps[:], 1.0)
                    nc.sync.dma_start(
                        out_d[m * P:(m + 1) * P, t * QT:(t + 1) * QT], st[:])
                    yield

            SC = 1.0 / np.sqrt(HD)

            pend = deque()    # ctx matmuls, lag 3 units (carried
            pend2 = deque()   # across pr/tile boundaries so forced
                              # flushes never pile up unready waiters)

            def flush_one():
                if pend2:
                    pend2.popleft()()
                if pend:
                    ctx_c, post_c = pend.popleft()
                    ctx_c()
                    if post_c is not None:
                        pend2.append(post_c)

            def flush_all():
                while pend or pend2:
                    flush_one()

            def attn_tile(t, pulls):
                # One unit per (key chunk, head). Scores take one PSUM
                # bank each from a 4-slot pool, so a unit's scores
                # conflict only with the exp four units back. ctx
                # matmuls lag three units so their es input is complete
                # before they reach the PE queue; normalize posts lag
                # one further unit.
                nkc = (t + 1) * QS
                for pr in range(MCH):
                    cps = {hh: psC.tile([P, QS, HD + 1], f32, tag="ctx",
                                        name=f"c{t}_{pr}_{hh}")
                           for hh in range(2)}
                    rec = {hh: work2.tile([P, QS], f32, tag="rc",
                                          name=f"rc{hh}", bufs=4)
                           for hh in range(2)}
                    tms = {}

                    def make_ctx(es, kc, hh, r, cps_=None):
                        cps_ = cps
                        head = 2 * pr + hh
                        jlo = 0 if r is None else r

                        def emit():
                            if r is not None:
                                nc.vector.tensor_mul(
                                    es[:, 0, KC * r:KC * r + KC],
                                    es[:, 0, KC * r:KC * r + KC], g[:])
                            for j in range(jlo, QS):
                                nc.tensor.matmul(
                                    cps_[hh][:, j, :],
                                    lhsT=es[:, 0, j * P:(j + 1) * P],
                                    rhs=v[kc][:, head, :],
                                    start=(kc == 0 and j == 0),
                                    stop=(r is not None and j == r),
                                    skip_group_check=True)
                        return emit

                    def make_ctx_pair(es, i0, hh, cps_=None):
                        cps_ = cps
                        head = 2 * pr + hh

                        def emit():
                            for h2 in range(2):
                                kc = i0 + h2
                                for j in range(QS):
                                    nc.tensor.matmul(
                                        cps_[hh][:, j, :],
                                        lhsT=es[:, h2,
                                                j * P:(j + 1) * P],
                                        rhs=v[kc][:, head, :],
                                        start=(kc == 0 and j == 0),
                                        stop=False,
                                        skip_group_check=True)
                        return emit

                    def make_post(hh, r, cps_=None, rec_=None, tms_=None,
                                  pr_=None):
                        cps_, rec_, tms_, pr_ = cps, rec, tms, pr

                        def post():
                            # qsub r of head (pr, hh) is complete:
                            # normalize (denominator at column HD); when
                            # both heads are done, ship the block back
                            # feature-major via the DMA transpose XBAR
                            if hh == 0:
                                tms_[r] = work2.tile([P, 2, HD], bf16,
                                                     tag="tm", name="tm",
                                                     bufs=6)
                            nc.vector.reciprocal_approx_fast(
                                rec_[hh][:, r:r + 1],
                                cps_[hh][:, r, HD:HD + 1])
                            nc.vector.tensor_scalar_mul(
                                tms_[r][:, hh, :], cps_[hh][:, r, 0:HD],
                                rec_[hh][:, r:r + 1])
                            if hh == 1:
                                nc.sync.dma_start_transpose(
                                    ctx[t][:, pr_, r * P:(r + 1) * P],
                                    tms_[r].rearrange("p a b -> p (a b)"))
                        return post

                    # non-diagonal chunks in pairs: two scores
                    # matmuls into one 2-bank slot, a single exp over
                    # both halves
                    for i0 in range(0, 4 * t, 2):
                        for hh in range(2):
                            off = HD * hh
                            sps = psSp.tile([P, 2, QT], f32, tag="sc",
                                            name="sps")
                            for h2 in range(2):
                                kc = i0 + h2
                                kcl, kco = divmod(kc * KC, QT)
                                nc.tensor.matmul(
                                    sps[:, h2, :],
                                    lhsT=kT[kcl][off:off + HD, pr,
                                                 kco:kco + KC],
                                    rhs=qT[t][off:off + HD, pr, :])
                            pulls()
                            es = work.tile([P, 2, QT], bf16, tag="e",
                                           name="es")
                            nc.scalar.activation(
                                es.rearrange("p a b -> p (a b)"),
                                sps.rearrange("p a b -> p (a b)"),
                                EXP, scale=SC)
                            pend.append((make_ctx_pair(es, i0, hh),
                                         None))
                            if len(pend) > 2:
                                flush_one()
                    # diagonal chunks: single, query-trimmed, masked
                    for r in range(QS):
                        kc = 4 * t + r
                        w0 = KC * r
                        kcl, kco = divmod(kc * KC, QT)
                        for hh in range(2):
                            off = HD * hh
                            sps = psSp.tile([P, 2, QT], f32, tag="sc",
                                            name="sps")
                            nc.tensor.matmul(
                                sps[:, 0, w0:],
                                lhsT=kT[kcl][off:off + HD, pr,
                                             kco:kco + KC],
                                rhs=qT[t][off:off + HD, pr, w0:])
                            pulls()
                            es = work.tile([P, 2, QT], bf16, tag="e",
                                           name="es")
                            nc.scalar.activation(es[:, 0, w0:],
                                                 sps[:, 0, w0:], EXP,
                                                 scale=SC)
                            pend.append((make_ctx(es, kc, hh, r),
                                         make_post(hh, r)))
                            if len(pend) > 2:
                                flush_one()

            N_QK_Q = MCH * 2 * (KCH + 1)
            N_V_Q = QS * (KCH + 1)

            def pull(bg, n):
                pulled = 0
                while n > 0 and bg:
                    try:
                        next(bg[0])
                        n -= 1
                        pulled += 1
                    except StopIteration:
                        bg.popleft()
                return pulled

            def phases():
                # Three filler queues keep the PE fed during the
                # latency-bound attention stretches:
                #   bgV  - V projection of the CURRENT tile's key chunks;
                #          hard deadline: before the tile's diagonal
                #          (singles) phase reads them
                #   bgQK - Q/K projection of the NEXT tile; deadline:
                #          end of the current tile
                #   bgO  - output projections; no deadline, so they are
                #          deferred to the last (most starved) tile
                bgV = deque()
                bgQK = deque()
                bgO = deque()
                remV = [0]
                remQK = [0]

                # prologue: just enough of proj(0) for att(0) pr=0 (the
                # m=0 half of Q/K plus v chunks 0-3); the m=1 half drains
                # via bgQK during att(0) well before pr=1 needs it
                for _ in gen_proj_qk(wq, qT, 0, 0):
                    pass
                for _ in gen_proj_qk(wk, kT, 0, 0):
                    pass
                for _ in gen_proj_v4(0):
                    pass

                def gen_proj0_m1():
                    yield from gen_proj_qk(wq, qT, 1, 0)
                    yield from gen_proj_qk(wk, kT, 1, 0)

                bgQK.append(gen_proj0_m1())
                remQK[0] += 2 * (KCH + 1)

                RATE_O = {0: 0, 1: 0, 2: 1, 3: 1}
                for t in range(NQT):
                    if t + 1 < NQT:
                        bgQK.append(gen_proj_qk_tile(t + 1))
                        remQK[0] += N_QK_Q
                    if t >= 1:
                        # V projection of THIS tile's diagonal key
                        # chunks: drains over the pr=0 pairs phase,
                        # finishing before the singles read it
                        bgV.append(gen_proj_v4(t))
                        remV[0] += N_V_Q
                    points = [MCH * 2 * (2 * t + QS)]
                    total = points[0]
                    vpts = [4 * t if t >= 1 else 2]

                    def pulls():
                        if vpts[0] > 0:
                            n = -(-remV[0] // vpts[0])
                        else:
                            n = remV[0]
                        n = min(n, remV[0])
                        remV[0] -= n
                        pull(bgV, n)
                        n = -(-remQK[0] // max(points[0], 1))
                        n = min(n, remQK[0])
                        remQK[0] -= n
                        pull(bgQK, n)
                        # outproj(t-1) reads ctx written by transposes
                        # that may still sit in the carried pipeline for
                        # the first few units of this tile; pulling bgO
                        # before they are emitted would hide the RAW dep
                        if total - points[0] > 6:
                            extra = 2 if (not bgV and not bgQK) else 0
                            pull(bgO, RATE_O[t] + extra)
                        points[0] -= 1
                        vpts[0] -= 1

                    attn_tile(t, pulls)
                    pull(bgV, 10 ** 9)   # drain: emission-order deps
                    pull(bgQK, 10 ** 9)
                    remV[0] = 0
                    remQK[0] = 0
                    bgO.append(gen_outproj(t))
                flush_all()
                pull(bgO, 10 ** 9)

            for _ in range(nreps):
                phases()

    nc.compile()
    return nc


def _mask():
    # mask[k, c] = 1.0 iff key row k <= query col c (within the
    # diagonal 128x128 band of a diagonal-crossing chunk)
    k = np.arange(P)[:, None]
    c = np.arange(KC)[None, :]
    return (k <= c).astype(np.float32)


def _in_maps(x, Wq, Wk, Wv, Wo):
    import ml_dtypes
    bf = ml_dtypes.bfloat16
    G = _mask().astype(bf)
    maps = []
    for c in range(NCORES):
        b, gidx = divmod(c, GROUPS)
        sl = slice(gidx * DG, (gidx + 1) * DG)
        maps.append({
            "xT": np.ascontiguousarray(x[b].T).astype(bf),
            "wq": np.ascontiguousarray(Wq[:, sl]).astype(bf),
            "wk": np.ascontiguousarray(Wk[:, sl]).astype(bf),
            "wv": np.ascontiguousarray(Wv[:, sl]).astype(bf),
            "wo": np.ascontiguousarray(Wo[sl, :]).astype(bf),
            "g": G,
        })
    return maps


def kernel(x, Wq, Wk, Wv, Wo, bo):
    global _compiled
    from concourse.bass_utils import run_bass_kernel_spmd

    x = np.asarray(x, dtype=np.float32)
    Wq = np.asarray(Wq, dtype=np.float32)
    Wk = np.asarray(Wk, dtype=np.float32)
    Wv = np.asarray(Wv, dtype=np.float32)
    Wo = np.asarray(Wo, dtype=np.float32)
    bo = np.asarray(bo, dtype=np.float32)

    if _compiled is None:
        _compiled = _build()
    nc = _compiled

    res = run_bass_kernel_spmd(nc, _in_maps(x, Wq, Wk, Wv, Wo),
                               list(range(NCORES)))
    out = np.zeros((B, S, D), dtype=np.float32)
    for c in range(NCORES):
        out[c // GROUPS] += res.results[c]["outT"].T.astype(np.float32)
    out += bo
    return out
